# revision 9
# baseline (speedup 1.0000x reference)
import os
import sys

import numpy as np

sys.path.insert(0, "/opt/trn_rl_repo")

# Problem constants (nn_AdditiveAttention): hardcoded per spec.
B, NQ, NK, D, DV, H = 4, 512, 512, 512, 512, 128
NCORES = 8
QPC = NQ // NCORES  # queries contributed by each batch to each core (64)

LAST_EXEC_NS = None
LAST_RESULT = {}


def _build_program(KPAD, L):
    """Build the SPMD Bass program. KPAD[b] = per-batch padded key count
    (multiple of 128), L[b] = true valid length. All cores run this one
    program; per-core data differences come only through in_maps."""
    import concourse.bass as bass
    import concourse.bacc as bacc
    import concourse.mybir as mybir
    from concourse.tile import TileContext

    f32 = mybir.dt.float32
    bf16 = mybir.dt.bfloat16
    OFF = np.concatenate([[0], np.cumsum(KPAD)]).astype(int)
    KSUM = int(OFF[-1])
    NQL = B * QPC  # local queries per core (256)

    nc = bacc.Bacc("TRN2", target_bir_lowering=False, debug=False)

    qt_d = nc.dram_tensor("qt", [D, NQL], f32, kind="ExternalInput")
    kt_d = nc.dram_tensor("kt", [D, KSUM], f32, kind="ExternalInput")
    v_d = nc.dram_tensor("v", [KSUM, DV], bf16, kind="ExternalInput")
    wq_d = nc.dram_tensor("wq", [D, H], f32, kind="ExternalInput")
    wk_d = nc.dram_tensor("wk", [D, H], f32, kind="ExternalInput")
    oneh_d = nc.dram_tensor("oneh", [H, 128 * 128], bf16, kind="ExternalInput")
    eye_d = nc.dram_tensor("eye", [128, QPC], bf16, kind="ExternalInput")
    out_d = nc.dram_tensor("out", [NQL, DV], f32, kind="ExternalOutput")

    Tanh = mybir.ActivationFunctionType.Tanh
    Exp = mybir.ActivationFunctionType.Exp
    AX = mybir.AxisListType.X

    with TileContext(nc) as tc:
        with (
            tc.tile_pool(name="const", bufs=1) as cpool,
            tc.tile_pool(name="proj", bufs=1) as projpool,
            tc.tile_pool(name="s", bufs=2) as spool,
            tc.tile_pool(name="t", bufs=2) as tpool,
            tc.tile_pool(name="p", bufs=2) as ppool,
            tc.tile_pool(name="stat", bufs=4) as statpool,
            tc.tile_pool(name="osb", bufs=2) as opool,
            tc.tile_pool(name="pps", bufs=2, space="PSUM") as projps,
            tc.tile_pool(name="sps", bufs=2, space="PSUM") as scorps,
            tc.tile_pool(name="ops", bufs=2, space="PSUM") as ops,
            tc.tile_pool(name="tps", bufs=2, space="PSUM") as tps,
        ):
            # ---- load constants
            qt_sb = []
            kt_sb = []
            wq_sb = []
            wk_sb = []
            for i in range(4):
                t = cpool.tile([128, NQL], f32, tag=f"qt{i}")
                nc.sync.dma_start(t[:], qt_d.rearrange("(n p) m -> n p m", p=128)[i])
                qt_sb.append(t)
                t = cpool.tile([128, KSUM], f32, tag=f"kt{i}")
                nc.sync.dma_start(t[:], kt_d.rearrange("(n p) m -> n p m", p=128)[i])
                kt_sb.append(t)
                t = cpool.tile([128, H], f32, tag=f"wq{i}")
                nc.sync.dma_start(t[:], wq_d.rearrange("(n p) m -> n p m", p=128)[i])
                wq_sb.append(t)
                t = cpool.tile([128, H], f32, tag=f"wk{i}")
                nc.sync.dma_start(t[:], wk_d.rearrange("(n p) m -> n p m", p=128)[i])
                wk_sb.append(t)
            v_sb = []
            for i in range(KSUM // 128):
                t = cpool.tile([128, DV], bf16, tag=f"v{i}")
                nc.sync.dma_start(t[:], v_d.rearrange("(n p) m -> n p m", p=128)[i])
                v_sb.append(t)
            oneh_sb = cpool.tile([128, 128 * 128], bf16, tag="oneh")
            for i in range(4):
                nc.sync.dma_start(
                    oneh_sb[:, i * 4096 : (i + 1) * 4096],
                    oneh_d[:, i * 4096 : (i + 1) * 4096],
                )
            eye_sb = cpool.tile([128, QPC], bf16, tag="eye")
            nc.sync.dma_start(eye_sb[:], eye_d[:])

            # ---- projections (f32): QpT (H x NQL), KpT (H x KSUM)
            qp_sb = projpool.tile([128, NQL], f32, tag="qp")
            qp_ps = projps.tile([128, 512], f32, tag="projps")
            for dc in range(4):
                nc.tensor.matmul(
                    qp_ps[:, :NQL], wq_sb[dc][:], qt_sb[dc][:],
                    start=(dc == 0), stop=(dc == 3),
                )
            nc.vector.tensor_copy(qp_sb[:], qp_ps[:, :NQL])

            kp_sb = projpool.tile([128, KSUM], f32, tag="kp")
            nkc = (KSUM + 511) // 512
            for kc in range(nkc):
                c0 = kc * 512
                cw = min(512, KSUM - c0)
                kp_ps = projps.tile([128, 512], f32, tag="projps")
                for dc in range(4):
                    nc.tensor.matmul(
                        kp_ps[:, :cw], wk_sb[dc][:], kt_sb[dc][:, c0 : c0 + cw],
                        start=(dc == 0), stop=(dc == 3),
                    )
                nc.vector.tensor_copy(kp_sb[:, c0 : c0 + cw], kp_ps[:, :cw])

            # ---- main phase: per q-block (128 queries = 2 batch groups)
            for qb in range(2):
                bs = [2 * qb, 2 * qb + 1]
                blockmax = max(KPAD[b] for b in bs)
                sc_ps = scorps.tile([128, blockmax], f32, tag="scores")

                # scores: tanh(Qp[q]+Kp) reduced against wv via one-hot matmuls
                for gi, b in enumerate(bs):
                    kpad = KPAD[b]
                    koff = int(OFF[b])
                    chunk = 64
                    while chunk * kpad > 6144:
                        chunk //= 2
                    for c in range(QPC // chunk):
                        fd = chunk * kpad
                        s_t = spool.tile([128, 6144], f32, tag="s")
                        for j in range(chunk):
                            q = b * QPC + c * chunk + j  # core-local query idx
                            nc.vector.tensor_scalar_add(
                                s_t[:, j * kpad : (j + 1) * kpad],
                                kp_sb[:, koff : koff + kpad],
                                qp_sb[:, q : q + 1],
                            )
                        t_t = tpool.tile([128, 6144], bf16, tag="t")
                        nc.scalar.activation(t_t[:, :fd], s_t[:, :fd], Tanh)
                        for j in range(chunk):
                            qib = gi * QPC + c * chunk + j  # 0..127 within block
                            first = qib == 0
                            last = qib == 127
                            n = blockmax if first else kpad
                            nc.tensor.matmul(
                                sc_ps[:, :n],
                                oneh_sb[:, qib * 128 : qib * 128 + 128],
                                t_t[:, j * kpad : j * kpad + n],
                                start=first, stop=last,
                            )

                # softmax + P@V per 64-row group
                o_ps = ops.tile([128, DV], f32, tag="ops")
                o_sb = opool.tile([128, DV], f32, tag="osb")
                for gi, b in enumerate(bs):
                    kpad = KPAD[b]
                    koff = int(OFF[b])
                    lb = int(L[b])
                    r0, r1 = gi * QPC, gi * QPC + QPC
                    if lb < kpad:
                        nc.vector.memset(sc_ps[r0:r1, lb:kpad], -1e20)
                    mx = statpool.tile([128, 1], f32, tag="mx")
                    nc.vector.reduce_max(mx[r0:r1, :], sc_ps[r0:r1, :kpad], axis=AX)
                    nmx = statpool.tile([128, 1], f32, tag="nmx")
                    nc.vector.tensor_scalar_mul(nmx[r0:r1, :], mx[r0:r1, :], -1.0)
                    p_t = ppool.tile([128, blockmax], bf16, tag="p")
                    ssum = statpool.tile([128, 1], f32, tag="ssum")
                    nc.scalar.activation(
                        p_t[r0:r1, :kpad], sc_ps[r0:r1, :kpad], Exp,
                        bias=nmx[r0:r1, :], accum_out=ssum[r0:r1, :],
                    )
                    rs = statpool.tile([128, 1], f32, tag="rs")
                    nc.vector.reciprocal(rs[r0:r1, :], ssum[r0:r1, :])
                    for kc in range(kpad // 128):
                        wt_ps = tps.tile([128, QPC], bf16, tag="wtps")
                        nc.tensor.transpose(
                            wt_ps[:],
                            p_t[r0:r1, kc * 128 : (kc + 1) * 128],
                            eye_sb[r0:r1, :],
                        )
                        wt_sb = statpool.tile([128, QPC], bf16, tag="wtsb")
                        nc.vector.tensor_copy(wt_sb[:], wt_ps[:])
                        nc.tensor.matmul(
                            o_ps[r0:r1, :],
                            wt_sb[:],
                            v_sb[(koff + kc * 128) // 128][:],
                            start=(kc == 0), stop=(kc == kpad // 128 - 1),
                        )
                    nc.vector.tensor_scalar_mul(
                        o_sb[r0:r1, :], o_ps[r0:r1, :], rs[r0:r1, :]
                    )
                nc.sync.dma_start(out_d[qb * 128 : (qb + 1) * 128, :], o_sb[:])

    nc.compile()
    return nc


def _install_profile_hook():
    """Register the NTFF profile hook that this container's antenv lacks,
    so run_bass_kernel_spmd(trace=True) can report exec_time_ns."""
    import types

    import antenv

    try:
        import antenv.axon_hooks  # noqa: F401
        return
    except ImportError:
        pass
    try:
        from trn_agent_boot.trn_boot import _ntff_profile_via_ctypes
    except ImportError:
        return
    hook = _ntff_profile_via_ctypes("/opt/axon/libaxon_pjrt.so")
    m = types.ModuleType("antenv.axon_hooks")
    m.get_axon_ntff_profile_hook = lambda: hook
    m.set_axon_ntff_profile_hook = lambda h: None
    sys.modules["antenv.axon_hooks"] = m
    antenv.axon_hooks = m


def kernel(Q, K, V, Wq, Wk, wv, valid_lens):
    global LAST_EXEC_NS
    import ml_dtypes
    from concourse.bass_utils import run_bass_kernel_spmd

    Q = np.asarray(Q, dtype=np.float32)
    K = np.asarray(K, dtype=np.float32)
    V = np.asarray(V, dtype=np.float32)
    Wq = np.asarray(Wq, dtype=np.float32)
    Wk = np.asarray(Wk, dtype=np.float32)
    wv = np.asarray(wv, dtype=np.float32)
    L = np.asarray(valid_lens).astype(np.int64)

    KPAD = [int(min(NK, -(-int(l) // 128) * 128)) for l in L]
    OFF = np.concatenate([[0], np.cumsum(KPAD)]).astype(int)
    KSUM = int(OFF[-1])

    nc = _build_program(KPAD, L)

    # shared tensors
    kt = np.ascontiguousarray(
        np.concatenate([K[b, : KPAD[b], :] for b in range(B)], axis=0).T
    )
    v16 = np.ascontiguousarray(
        np.concatenate([V[b, : KPAD[b], :] for b in range(B)], axis=0)
    ).astype(ml_dtypes.bfloat16)
    oneh3 = np.zeros((H, 128, 128), dtype=ml_dtypes.bfloat16)
    oneh3[:, np.arange(128), np.arange(128)] = wv[:, None].astype(ml_dtypes.bfloat16)
    oneh = oneh3.reshape(H, 128 * 128)
    eye = np.concatenate(
        [np.eye(QPC, dtype=ml_dtypes.bfloat16)] * (128 // QPC), axis=0
    )

    in_maps = []
    for c in range(NCORES):
        qloc = np.concatenate(
            [Q[b, c * QPC : (c + 1) * QPC, :] for b in range(B)], axis=0
        )  # (256, 512)
        in_maps.append(
            {
                "qt": np.ascontiguousarray(qloc.T),
                "kt": kt,
                "v": v16,
                "wq": Wq,
                "wk": Wk,
                "oneh": oneh,
                "eye": eye,
            }
        )

    trace = os.environ.get("KERNEL_PROFILE", "0") == "1"
    if trace:
        _install_profile_hook()
    res = run_bass_kernel_spmd(nc, in_maps, list(range(NCORES)), trace=trace)
    LAST_EXEC_NS = res.exec_time_ns
    LAST_RESULT["res"] = res

    out = np.empty((B, NQ, DV), dtype=np.float32)
    for c in range(NCORES):
        o = np.asarray(res.results[c]["out"])
        for b in range(B):
            out[b, c * QPC : (c + 1) * QPC, :] = o[b * QPC : (b + 1) * QPC, :]
    return out


# revision 10
# speedup vs baseline: 1.3222x; 1.3222x over previous
import os
import sys

import numpy as np

sys.path.insert(0, "/opt/trn_rl_repo")

# Problem constants (nn_AdditiveAttention): hardcoded per spec.
B, NQ, NK, D, DV, H = 4, 512, 512, 512, 512, 128
NCORES = 8
QPC = NQ // NCORES  # queries contributed by each batch to each core (64)

LAST_EXEC_NS = None
LAST_RESULT = {}


def _plan(valid_lens):
    L = [int(x) for x in np.asarray(valid_lens).reshape(-1)]
    KP32 = [min(NK, -(-l // 32) * 32) for l in L]   # add/tanh/scores extent
    KPV = [min(NK, -(-l // 128) * 128) for l in L]  # PV (128-aligned) extent
    return L, KP32, KPV


def _build_program(L, KP32, KPV):
    """Build the SPMD Bass program. All cores run this one program;
    per-core data differences come only through in_maps."""
    import concourse.bacc as bacc
    import concourse.mybir as mybir
    from concourse.tile import TileContext

    f32 = mybir.dt.float32
    bf16 = mybir.dt.bfloat16
    OFF32 = np.concatenate([[0], np.cumsum(KP32)]).astype(int)
    OFFV = np.concatenate([[0], np.cumsum(KPV)]).astype(int)
    KSUM32 = int(OFF32[-1])
    KSUMV = int(OFFV[-1])
    NQL = B * QPC  # local queries per core (256)

    nc = bacc.Bacc("TRN2", target_bir_lowering=False, debug=False)

    qt_d = nc.dram_tensor("qt", [D, NQL], f32, kind="ExternalInput")
    kt_d = nc.dram_tensor("kt", [D, KSUM32], f32, kind="ExternalInput")
    v_d = nc.dram_tensor("v", [KSUMV, DV], bf16, kind="ExternalInput")
    wq_d = nc.dram_tensor("wq", [D, H], f32, kind="ExternalInput")
    wk_d = nc.dram_tensor("wk", [D, H], f32, kind="ExternalInput")
    oneh_d = nc.dram_tensor("oneh", [H, 128 * 128], bf16, kind="ExternalInput")
    eye_d = nc.dram_tensor("eye", [128, QPC], bf16, kind="ExternalInput")
    out_d = nc.dram_tensor("out", [NQL, DV], f32, kind="ExternalOutput")

    Tanh = mybir.ActivationFunctionType.Tanh
    Exp = mybir.ActivationFunctionType.Exp
    Copy = mybir.ActivationFunctionType.Copy
    AX = mybir.AxisListType.X

    # per-batch query chunking for the s/t pipeline tiles
    SMAX = 6144
    CHUNK = {}
    for b in range(B):
        c = 64
        while c * KP32[b] > SMAX:
            c //= 2
        CHUNK[b] = c

    with TileContext(nc) as tc:
        with (
            tc.tile_pool(name="const", bufs=1) as cpool,
            tc.tile_pool(name="proj", bufs=1) as projpool,
            tc.tile_pool(name="s", bufs=3) as spool,
            tc.tile_pool(name="t", bufs=3) as tpool,
            tc.tile_pool(name="p", bufs=2) as ppool,
            tc.tile_pool(name="stat", bufs=4) as statpool,
            tc.tile_pool(name="osb", bufs=2) as opool,
            tc.tile_pool(name="pps", bufs=2, space="PSUM") as projps,
            tc.tile_pool(name="sps", bufs=2, space="PSUM") as scorps,
            tc.tile_pool(name="ops", bufs=2, space="PSUM") as ops,
            tc.tile_pool(name="tps", bufs=2, space="PSUM") as tps,
        ):
            # ---- load constants
            qt_sb = []
            kt_sb = []
            wq_sb = []
            wk_sb = []
            for i in range(4):
                t = cpool.tile([128, NQL], f32, tag=f"qt{i}")
                nc.sync.dma_start(t[:], qt_d.rearrange("(n p) m -> n p m", p=128)[i])
                qt_sb.append(t)
                t = cpool.tile([128, KSUM32], f32, tag=f"kt{i}")
                nc.sync.dma_start(t[:], kt_d.rearrange("(n p) m -> n p m", p=128)[i])
                kt_sb.append(t)
                t = cpool.tile([128, H], f32, tag=f"wq{i}")
                nc.sync.dma_start(t[:], wq_d.rearrange("(n p) m -> n p m", p=128)[i])
                wq_sb.append(t)
                t = cpool.tile([128, H], f32, tag=f"wk{i}")
                nc.sync.dma_start(t[:], wk_d.rearrange("(n p) m -> n p m", p=128)[i])
                wk_sb.append(t)
            v_sb = []
            for i in range(KSUMV // 128):
                t = cpool.tile([128, DV], bf16, tag=f"v{i}")
                nc.sync.dma_start(t[:], v_d.rearrange("(n p) m -> n p m", p=128)[i])
                v_sb.append(t)
            oneh_sb = cpool.tile([128, 128 * 128], bf16, tag="oneh")
            for i in range(4):
                nc.sync.dma_start(
                    oneh_sb[:, i * 4096 : (i + 1) * 4096],
                    oneh_d[:, i * 4096 : (i + 1) * 4096],
                )
            eye_sb = cpool.tile([128, QPC], bf16, tag="eye")
            nc.sync.dma_start(eye_sb[:], eye_d[:])

            # ---- projections (f32 in, QpT f32 / KpT bf16 out)
            qp_sb = projpool.tile([128, NQL], f32, tag="qp")
            qp_ps = projps.tile([128, 512], f32, tag="projps")
            for dc in range(4):
                nc.tensor.matmul(
                    qp_ps[:, :NQL], wq_sb[dc][:], qt_sb[dc][:],
                    start=(dc == 0), stop=(dc == 3),
                )
            nc.vector.tensor_copy(qp_sb[:], qp_ps[:, :NQL])

            kp_sb = projpool.tile([128, KSUM32], bf16, tag="kp")
            nkc = (KSUM32 + 511) // 512
            for kc in range(nkc):
                c0 = kc * 512
                cw = min(512, KSUM32 - c0)
                kp_ps = projps.tile([128, 512], f32, tag="projps")
                for dc in range(4):
                    nc.tensor.matmul(
                        kp_ps[:, :cw], wk_sb[dc][:], kt_sb[dc][:, c0 : c0 + cw],
                        start=(dc == 0), stop=(dc == 3),
                    )
                nc.vector.tensor_copy(kp_sb[:, c0 : c0 + cw], kp_ps[:, :cw])

            # ---- main phase: per q-block (128 queries = 2 batch groups)
            for qb in range(2):
                bs = [2 * qb, 2 * qb + 1]
                blockmax = max(KP32[b] for b in bs)
                sc_ps = scorps.tile([128, blockmax], f32, tag="scores")

                # scores: tanh(Qp[q]+Kp) reduced against wv via one-hot matmuls
                for gi, b in enumerate(bs):
                    kpad = KP32[b]
                    koff = int(OFF32[b])
                    chunk = CHUNK[b]
                    for c in range(QPC // chunk):
                        s_t = spool.tile([128, SMAX], bf16, tag="s")
                        for j in range(chunk):
                            q = b * QPC + c * chunk + j  # core-local query idx
                            nc.vector.tensor_scalar_add(
                                s_t[:, j * kpad : (j + 1) * kpad],
                                kp_sb[:, koff : koff + kpad],
                                qp_sb[:, q : q + 1],
                            )
                        fd = chunk * kpad
                        t_t = tpool.tile([128, SMAX], bf16, tag="t")
                        nc.scalar.activation(t_t[:, :fd], s_t[:, :fd], Tanh)
                        for j in range(chunk):
                            qib = gi * QPC + c * chunk + j  # 0..127 within block
                            first = qib == 0
                            last = qib == 127
                            n = blockmax if first else kpad
                            nc.tensor.matmul(
                                sc_ps[:, :n],
                                oneh_sb[:, qib * 128 : qib * 128 + 128],
                                t_t[:, j * kpad : j * kpad + n],
                                start=first, stop=last,
                            )

                # softmax + P@V per 64-row group
                o_ps = ops.tile([128, DV], f32, tag="ops")
                o_sb = opool.tile([128, DV], f32, tag="osb")
                for gi, b in enumerate(bs):
                    kpadv = KPV[b]
                    koffv = int(OFFV[b])
                    lb = L[b]
                    r0, r1 = gi * QPC, gi * QPC + QPC
                    nmx = statpool.tile([128, 1], f32, tag="nmx")
                    nc.vector.reduce_max(
                        nmx[r0:r1, :], sc_ps[r0:r1, :lb], axis=AX, negate=True
                    )
                    p_t = ppool.tile([128, 512], bf16, tag="p")
                    ssum = statpool.tile([128, 1], f32, tag="ssum")
                    nc.scalar.activation(
                        p_t[r0:r1, :lb], sc_ps[r0:r1, :lb], Exp,
                        bias=nmx[r0:r1, :], accum_out=ssum[r0:r1, :],
                    )
                    if lb < kpadv:
                        nc.vector.memset(p_t[r0:r1, lb:kpadv], 0.0)
                    rs = statpool.tile([128, 1], f32, tag="rs")
                    nc.vector.reciprocal(rs[r0:r1, :], ssum[r0:r1, :])
                    for kc in range(kpadv // 128):
                        wt_ps = tps.tile([128, QPC], bf16, tag="wtps")
                        nc.tensor.transpose(
                            wt_ps[:],
                            p_t[r0:r1, kc * 128 : (kc + 1) * 128],
                            eye_sb[r0:r1, :],
                        )
                        wt_sb = statpool.tile([128, QPC], bf16, tag="wtsb")
                        nc.vector.tensor_copy(wt_sb[:], wt_ps[:])
                        nc.tensor.matmul(
                            o_ps[r0:r1, :],
                            wt_sb[:],
                            v_sb[(koffv + kc * 128) // 128][:],
                            start=(kc == 0), stop=(kc == kpadv // 128 - 1),
                        )
                    nc.scalar.activation(
                        o_sb[r0:r1, :], o_ps[r0:r1, :], Copy, scale=rs[r0:r1, :]
                    )
                nc.sync.dma_start(out_d[qb * 128 : (qb + 1) * 128, :], o_sb[:])

    nc.compile()
    return nc


def _install_profile_hook():
    """Register the NTFF profile hook that this container's antenv lacks,
    so run_bass_kernel_spmd(trace=True) can report exec_time_ns."""
    import types

    import antenv

    try:
        import antenv.axon_hooks  # noqa: F401
        return
    except ImportError:
        pass
    try:
        from trn_agent_boot.trn_boot import _ntff_profile_via_ctypes
    except ImportError:
        return
    hook = _ntff_profile_via_ctypes("/opt/axon/libaxon_pjrt.so")
    m = types.ModuleType("antenv.axon_hooks")
    m.get_axon_ntff_profile_hook = lambda: hook
    m.set_axon_ntff_profile_hook = lambda h: None
    sys.modules["antenv.axon_hooks"] = m
    antenv.axon_hooks = m


def kernel(Q, K, V, Wq, Wk, wv, valid_lens):
    global LAST_EXEC_NS
    import ml_dtypes
    from concourse.bass_utils import run_bass_kernel_spmd

    Q = np.asarray(Q, dtype=np.float32)
    K = np.asarray(K, dtype=np.float32)
    V = np.asarray(V, dtype=np.float32)
    Wq = np.asarray(Wq, dtype=np.float32)
    Wk = np.asarray(Wk, dtype=np.float32)
    wv = np.asarray(wv, dtype=np.float32)

    L, KP32, KPV = _plan(valid_lens)
    nc = _build_program(L, KP32, KPV)

    # shared tensors
    kt = np.ascontiguousarray(
        np.concatenate([K[b, : KP32[b], :] for b in range(B)], axis=0).T
    )
    v16 = np.ascontiguousarray(
        np.concatenate([V[b, : KPV[b], :] for b in range(B)], axis=0)
    ).astype(ml_dtypes.bfloat16)
    oneh3 = np.zeros((H, 128, 128), dtype=ml_dtypes.bfloat16)
    oneh3[:, np.arange(128), np.arange(128)] = wv[:, None].astype(ml_dtypes.bfloat16)
    oneh = oneh3.reshape(H, 128 * 128)
    eye = np.concatenate(
        [np.eye(QPC, dtype=ml_dtypes.bfloat16)] * (128 // QPC), axis=0
    )

    in_maps = []
    for c in range(NCORES):
        qloc = np.concatenate(
            [Q[b, c * QPC : (c + 1) * QPC, :] for b in range(B)], axis=0
        )  # (256, 512)
        in_maps.append(
            {
                "qt": np.ascontiguousarray(qloc.T),
                "kt": kt,
                "v": v16,
                "wq": Wq,
                "wk": Wk,
                "oneh": oneh,
                "eye": eye,
            }
        )

    trace = os.environ.get("KERNEL_PROFILE", "0") == "1"
    if trace:
        _install_profile_hook()
    res = run_bass_kernel_spmd(nc, in_maps, list(range(NCORES)), trace=trace)
    LAST_EXEC_NS = res.exec_time_ns
    LAST_RESULT["res"] = res

    out = np.empty((B, NQ, DV), dtype=np.float32)
    for c in range(NCORES):
        o = np.asarray(res.results[c]["out"])
        for b in range(B):
            out[b, c * QPC : (c + 1) * QPC, :] = o[b * QPC : (b + 1) * QPC, :]
    return out
